# revision 57
# baseline (speedup 1.0000x reference)
"""Trainium2 Bass kernel for nn_Block (attention + MoE routing block), fp8.

Contract: kernel(**inputs) takes FULL unsharded inputs, returns FULL output.
Internally: two SPMD launches over 8 NeuronCores.
  Launch A: attention, tensor-parallel over heads (2 heads per core).
            Single interleaved stream: the softmax exp on ACT is the serial
            floor (~133us); QKV+rope, scores, AV, WO are all scheduled to
            hide under it (rolling start on batch 0, deferred epilogue/WO).
  Launch B: MoE, expert-parallel (1 routed expert per core) + data-parallel
            shared expert; host routing between launches. fp8 DoubleRow for
            all three matmul stages, ACT Silu.
"""

import numpy as np
import ml_dtypes

E4NP = ml_dtypes.float8_e4m3
BF16NP = ml_dtypes.bfloat16

# ---- problem shapes (hardcoded per contract) ----
B, S, D, H, HD = 2, 2048, 1024, 16, 64
E, TOPK = 8, 2
HM = 1024   # moe inter dim
HS = 1024   # shared expert hidden
N = B * S   # 4096 tokens
NCORES = 8
HPC = H // NCORES          # heads per core = 2
CAP = 1152                 # routed-token capacity per expert
SHARE = N // NCORES        # shared-expert tokens per core = 512
EPS = 1e-6
FP32 = np.float32
KCB = D // 128   # 8 contraction blocks
HCB = HM // 128  # 8 inter blocks
VW = 80          # per-head v block width (64 dims + 1 ones + pad)

# fp8 scale plan (pow2). Device-value scales fixed; weight scales computed
# per call on host, dequants ride in via input APs.
SQ8 = 32.0                 # q rope-out fp8 scale
SV8 = 32.0                 # v fp8 scale
SXF = 2.0                  # moe input fp8 scale
SW3 = 8.0                  # moe w3 fp8 scale; SXF*SW3=16 keeps h13 in fp8 range

_CACHE = {}


def _p2(target, amax):
    if amax <= 0:
        return 1.0
    return float(2.0 ** np.floor(np.log2(target / amax)))


# --------------------------------------------------------------------------
# device programs
# --------------------------------------------------------------------------

def _mk_bass():
    from concourse import bacc
    return bacc.Bacc(
        "TRN2",
        target_bir_lowering=False,
        debug=False,
        enable_asserts=True,
        num_devices=NCORES,
    )


def _build_attn():
    """Launch A: per-core attention for 2 heads; outputs partial (N, D) of @wo.

    One interleaved stream.  ACT runs only the 128 softmax exps ([128,1024]
    each, ~133us total) -- everything else hides under them:
      - prologue: QKV+rope for token chunks 0..2 (batch 0 mostly)
      - 8 instances of (batch, 512-query chunk): per kcp (256 keys) 2 score
        psums -> 2 exps -> 2 AV DoubleRow accums
      - chunks 3..7 QKV+rope interleaved into instance kcp slots
      - softmax epilogue + WO of instance i deferred into instance i+1
    """
    import concourse.mybir as mybir
    import concourse.tile as tile
    from concourse.bass import ts

    dt = mybir.dt.float32
    dtr = mybir.dt.float32r
    f8 = mybir.dt.float8e4
    bf = mybir.dt.bfloat16
    f16 = mybir.dt.float16
    DR = mybir.MatmulPerfMode.DoubleRow
    Exp = mybir.ActivationFunctionType.Exp
    Copy = mybir.ActivationFunctionType.Copy
    nc = _mk_bass()

    xnt = nc.dram_tensor("xnt", [D, N], f8, kind="ExternalInput").ap()
    # startup pack: [x0(kc0-3) | wq | wqs | x0(kc4-7) | wk | wks | wv] so the
    # first-chunk critical chain is two DMAs
    spk = nc.dram_tensor("spk", [128, 9216], f8, kind="ExternalInput").ap()
    wo2 = nc.dram_tensor("wo2", [64, 2048], f8, kind="ExternalInput").ap()
    # cos|sin packed -> two DMAs (first 512 cols, then the rest)
    cspack = nc.dram_tensor("cspack", [128, 2 * S], bf,
                            kind="ExternalInput").ap()
    onesv = nc.dram_tensor("onesv", [1, 64], dtr, kind="ExternalInput").ap()
    # sconst cols: 0=exp scale, 1=v requant, 2=partial dequant
    sconst = nc.dram_tensor("sconst", [128, 3], dt, kind="ExternalInput").ap()
    partial = nc.dram_tensor("partial", [N, D], f16, kind="ExternalOutput").ap()

    xnt_r = xnt.rearrange("(kc p) t -> kc p t", p=128)        # 8 x 128 x 4096
    partial_r2 = partial.rearrange("(g two p) d -> g p two d", p=128, two=2)

    KC = D // 128      # 8 contraction chunks

    with tile.TileContext(nc) as tc:
        with tc.tile_pool(name="persist", bufs=1) as pp:
            qT = pp.tile([128, N], f8, tag="qT")
            kT = pp.tile([128, N], f8, tag="kT")
            cs_b = pp.tile([128, 2 * S], bf, tag="cs")
            cos_t = cs_b[:, 0:S]
            sin_t = cs_b[:, S:2 * S]
            aout = pp.tile([64, 2 * N], f8, tag="aout")     # [h0 | h1] blocks
            ones_t = pp.tile([1, 64], dtr, tag="ones")
            scl_t = pp.tile([128, 3], dt, tag="scl")
            swall = pp.tile([128, 9216], f8, tag="swall")
            wq_b = swall[:, 2048:3072]
            wqs_b = swall[:, 3072:4096]
            wk_b = swall[:, 6144:7168]
            wks_b = swall[:, 7168:8192]
            wv_b = swall[:, 8192:9216]
            wo_b = pp.tile([64, 2048], f8, tag="wob")
            v_big = pp.tile([128, 32 * 2 * VW], f8, tag="vbig")

            def w3v(t):
                return [t[:, kc2 * 256:(kc2 + 1) * 256].rearrange(
                    "p (two m) -> p two m", two=2) for kc2 in range(KC // 2)]

            wq3, wqs3 = w3v(wq_b), w3v(wqs_b)
            wk3, wks3 = w3v(wk_b), w3v(wks_b)
            wv3 = w3v(wv_b)
            wo3 = wo_b[:, :].rearrange("p (two d) -> p two d", two=2)
            aout3 = aout[:, :].rearrange("p (two t) -> p two t", two=2)

            with (
                tc.tile_pool(name="xnstream", bufs=4) as xp,
                tc.tile_pool(name="ropetmp", bufs=4) as rp,
                tc.tile_pool(name="attn", bufs=2) as ap_,
                tc.tile_pool(name="oout", bufs=4) as op_,
                tc.tile_pool(name="norm", bufs=2) as np_,
                tc.tile_pool(name="ps_sc", bufs=2, space="PSUM") as scp,
                tc.tile_pool(name="ps_av", bufs=1, space="PSUM") as pav,
                tc.tile_pool(name="ps_misc", bufs=2, space="PSUM") as p3,
            ):
                # ---------- emission helpers ----------
                def emit_xt(tcix):
                    xt = xp.tile([128, 4096], f8, tag="xn", name=f"xn{tcix}")
                    nc.sync.dma_start(
                        out=xt[:, :].rearrange("p (kc t) -> p kc t", kc=KC),
                        in_=xnt_r.transpose([1, 0, 2])[:, :, ts(tcix, 512)])
                    return xt

                def x3view(xt):
                    return [xt[:, kc2 * 1024:(kc2 + 1) * 1024].rearrange(
                        "p (two t) -> p two t", two=2) for kc2 in range(KC // 2)]

                def rope_out(pc_lo, pc_hi, dstT, tcix, halves=1):
                    c0 = (tcix * 512) % S
                    hw = 512 // halves
                    for hf in range(halves):
                        o = hf * hw
                        a = rp.tile([128, hw], dtr, tag="ra",
                                    name=f"ra{tcix}_{id(pc_lo)%97}_{hf}")
                        bb = rp.tile([128, hw], dtr, tag="rb",
                                     name=f"rb{tcix}_{id(pc_lo)%97}_{hf}")
                        nc.vector.tensor_mul(a, pc_lo[:, o:o + hw],
                                             cos_t[:, c0 + o:c0 + o + hw])
                        nc.vector.tensor_mul(bb, pc_hi[:, o:o + hw],
                                             sin_t[:, c0 + o:c0 + o + hw])
                        dsl = dstT[:, tcix * 512 + o:tcix * 512 + o + hw]
                        if tcix <= 1:
                            # startup chain: DVE sub is lower-latency
                            nc.vector.tensor_sub(dsl, a, bb)
                        else:
                            nc.gpsimd.tensor_sub(dsl, a, bb)

                def emit_v(tcix, x3s):
                    pv = p3.tile([128, 512], dt, tag="pb", name=f"pv{tcix}")
                    for i in range(4):
                        for kc2 in range(KC // 2):
                            nc.tensor.matmul(
                                pv[:, ts(i, 128)],
                                x3s[kc2][:, :, ts(i, 128)], wv3[kc2],
                                start=(kc2 == 0), stop=(kc2 == KC // 2 - 1),
                                perf_mode=DR)
                    for i in range(4):
                        tt = tcix * 4 + i
                        base = tt * 2 * VW
                        dst3 = v_big[:, base:base + 2 * VW].rearrange(
                            "p (two f) -> p two f", two=2)[:, :, 0:64]
                        src3 = pv[:, ts(i, 128)].rearrange(
                            "p (two f) -> p two f", two=2)
                        nc.vector.tensor_scalar_mul(dst3, src3, scl_t[:, 1:2])

                def emit_chunk_big(tcix, x3s):
                    # prologue variant: q|qs and k|ks in [128,1024] sc-pool
                    pqc = scp.tile([128, 1024], dt, tag="sc", name=f"pqc{tcix}")
                    for kc2 in range(KC // 2):
                        st, sp = kc2 == 0, kc2 == KC // 2 - 1
                        nc.tensor.matmul(pqc[:, 0:512], wq3[kc2], x3s[kc2],
                                         start=st, stop=sp, perf_mode=DR)
                        nc.tensor.matmul(pqc[:, 512:1024], wqs3[kc2], x3s[kc2],
                                         start=st, stop=sp, perf_mode=DR)
                    rope_out(pqc[:, 0:512], pqc[:, 512:1024], qT, tcix)
                    pkc = scp.tile([128, 1024], dt, tag="sc", name=f"pkc{tcix}")
                    for kc2 in range(KC // 2):
                        st, sp = kc2 == 0, kc2 == KC // 2 - 1
                        nc.tensor.matmul(pkc[:, 0:512], wk3[kc2], x3s[kc2],
                                         start=st, stop=sp, perf_mode=DR)
                        nc.tensor.matmul(pkc[:, 512:1024], wks3[kc2], x3s[kc2],
                                         start=st, stop=sp, perf_mode=DR)
                    # chunk 0: produce kT in 256-token halves so the first
                    # score matmul starts one DVE-mul earlier
                    rope_out(pkc[:, 0:512], pkc[:, 512:1024], kT, tcix,
                             halves=2 if tcix == 0 else 1)
                    emit_v(tcix, x3s)

                def emit_chunk_q(tcix, xt):
                    x3s = x3view(xt)
                    pq = p3.tile([128, 512], dt, tag="pb", name=f"pq{tcix}")
                    for kc2 in range(KC // 2):
                        nc.tensor.matmul(pq, wq3[kc2], x3s[kc2],
                                         start=(kc2 == 0),
                                         stop=(kc2 == KC // 2 - 1), perf_mode=DR)
                    pqs = p3.tile([128, 512], dt, tag="pb", name=f"pqs{tcix}")
                    for kc2 in range(KC // 2):
                        nc.tensor.matmul(pqs, wqs3[kc2], x3s[kc2],
                                         start=(kc2 == 0),
                                         stop=(kc2 == KC // 2 - 1), perf_mode=DR)
                    rope_out(pq, pqs, qT, tcix)

                def emit_chunk_k(tcix, xt):
                    x3s = x3view(xt)
                    pk = p3.tile([128, 512], dt, tag="pb", name=f"pk{tcix}")
                    for kc2 in range(KC // 2):
                        nc.tensor.matmul(pk, wk3[kc2], x3s[kc2],
                                         start=(kc2 == 0),
                                         stop=(kc2 == KC // 2 - 1), perf_mode=DR)
                    pks = p3.tile([128, 512], dt, tag="pb", name=f"pks{tcix}")
                    for kc2 in range(KC // 2):
                        nc.tensor.matmul(pks, wks3[kc2], x3s[kc2],
                                         start=(kc2 == 0),
                                         stop=(kc2 == KC // 2 - 1), perf_mode=DR)
                    rope_out(pk, pks, kT, tcix)

                ot_box = {}

                def emit_wo(q0p, i, mix=False):
                    tt = q0p // 128 + i
                    ot = op_.tile([128, 1024], f16, tag="ot", name=f"ot{tt}")
                    lh3 = aout3[:, :, tt * 128:(tt + 1) * 128]
                    for j in range(2):
                        if mix and i >= 2:
                            # tail: av psum slots are free; 4 po tiles in
                            # flight lets PE/DVE/ACT evac run in parallel
                            po = pav.tile([128, 512], dt, tag=f"av{j}",
                                          name=f"po{tt}_{j}")
                        else:
                            po = p3.tile([128, 512], dt, tag="pb",
                                         name=f"po{tt}_{j}")
                        nc.tensor.matmul(po, lh3, wo3[:, :, ts(j, 512)],
                                         start=True, stop=True, perf_mode=DR)
                        osl = ot[:, j * 512:j * 512 + 512]
                        if mix and i >= 2:
                            nc.scalar.activation(osl, po, Copy,
                                                 scale=scl_t[:, 2:3])
                        else:
                            nc.vector.tensor_scalar_mul(osl, po,
                                                        scl_t[:, 2:3])
                    nc.sync.dma_start(out=partial_r2[tt // 2][:, tt % 2, :],
                                      in_=ot)

                epi_box = {}

                def emit_epi_a(inst, av, last=False):
                    # denominator reciprocal + numerator copy (DVE; ACT when
                    # the exp stream is over).  In the tail, interleave so
                    # ACT and DVE touch different av banks concurrently.
                    rden, avs = {}, {}
                    for h in range(HPC):
                        rden[h] = np_.tile([1, 512], dtr, tag=f"rden{h}",
                                           name=f"rden{h}_{inst}")
                        avs[h] = np_.tile([64, 512], dtr, tag=f"avs{h}",
                                          name=f"avs{h}_{inst}")
                    if last:
                        with nc.allow_low_precision("softmax denom recip"):
                            nc.vector.reciprocal(rden[0], av[0][64:65, :])
                            nc.scalar.activation(avs[1], av[1][0:64, :], Copy)
                            nc.vector.reciprocal(rden[1], av[1][64:65, :])
                            nc.scalar.activation(avs[0], av[0][0:64, :], Copy)
                    else:
                        for h in range(HPC):
                            with nc.allow_low_precision("softmax denom recip"):
                                nc.vector.reciprocal(rden[h], av[h][64:65, :])
                            nc.vector.tensor_copy(avs[h], av[h][0:64, :])
                    epi_box[inst] = [(rden[h], avs[h]) for h in range(HPC)]

                def emit_epi_b(inst, q0):
                    # broadcast recip via ones-matmul + aout write; bc lives
                    # in the misc psum pool so a late DVE recip never blocks
                    # the PE score stream (emitted 2 kcps after part A).
                    for h in range(HPC):
                        rden, avs = epi_box[inst][h]
                        bc = p3.tile([128, 512], dt, tag="pb",
                                     name=f"bc{h}_{inst}")
                        nc.tensor.matmul(bc[0:64, :], ones_t, rden,
                                         start=True, stop=True)
                        nc.vector.tensor_mul(
                            aout[0:64, h * N + q0: h * N + q0 + 512], avs,
                            bc[0:64, :])

                # ---------- initial DMAs (priority order on SP queue) ----------
                # chunk-0 critical chain first: xt0 low half, q weights,
                # first 512 cols of cos/sin, then the rest.
                xts = {}
                cs2 = cs_b[:, :].rearrange("p (two s) -> p two s", two=2)
                csp2 = cspack.rearrange("p (two s) -> p two s", two=2)
                nc.sync.dma_start(out=ones_t, in_=onesv)
                nc.sync.dma_start(out=swall[:, 0:4096], in_=spk[:, 0:4096])
                nc.sync.dma_start(out=swall[:, 4096:6144],
                                  in_=spk[:, 4096:6144])
                nc.sync.dma_start(out=cs2[:, :, 0:512], in_=csp2[:, :, 0:512])
                nc.sync.dma_start(out=swall[:, 6144:8192],
                                  in_=spk[:, 6144:8192])
                nc.sync.dma_start(out=swall[:, 8192:9216],
                                  in_=spk[:, 8192:9216])
                nc.sync.dma_start(out=scl_t, in_=sconst)
                nc.sync.dma_start(out=cs2[:, :, 512:S], in_=csp2[:, :, 512:S])
                xts[1] = emit_xt(1)
                nc.sync.dma_start(out=wo_b, in_=wo2)
                xts[2] = emit_xt(2)
                # chunk-0 x data lives inside the startup pack
                x3s0 = [swall[:, 0:2048][:, kc2 * 1024:(kc2 + 1) * 1024]
                        .rearrange("p (two t) -> p two t", two=2)
                        for kc2 in range(2)] + [
                        swall[:, 4096:6144][:, kc2 * 1024:(kc2 + 1) * 1024]
                        .rearrange("p (two t) -> p two t", two=2)
                        for kc2 in range(2)]
                # PE p-state warm-up: harmless dummy matmuls while the input
                # DMAs stream in, so chunk-0 projections run at full clock
                for wi in range(12):
                    pw = p3.tile([128, 512], dt, tag="pb", name=f"warm{wi}")
                    nc.tensor.matmul(pw[0:64, 0:64], ones_t, ones_t,
                                     start=True, stop=True)
                nc.gpsimd.memset(v_big[:, 64::VW], 1.0)
                # preload the Exp activation table while ACT is idle
                warm = np_.tile([1, 3], dtr, tag="warm")
                nc.scalar.activation(warm, scl_t[0:1, 0:3], Exp)

                # ---------- prologue: chunk 0 only ----------
                emit_chunk_big(0, x3s0)
                xts[3] = emit_xt(3)

                # ---------- instances ----------
                # instance i handles (b, qc) = divmod(i, 4).
                # inst0 absorbs chunks 1..3 (split q/k/v); inst1..4 get 4..7.
                # only K and V of chunks 1..3 gate instance 0; Q projections
                # are deferred to late kcp slots (first needed by inst 1).
                chunk_sched = {
                    (0, 0): [lambda: emit_chunk_k(1, xts[1])],
                    (0, 1): [lambda: emit_v(1, x3view(xts[1])),
                             lambda: emit_chunk_k(2, xts[2])],
                    (0, 2): [lambda: emit_v(2, x3view(xts[2]))],
                    (0, 3): [lambda: emit_chunk_k(3, xts[3])],
                    (0, 4): [lambda: emit_v(3, x3view(xts[3]))],
                    (0, 5): [lambda: emit_chunk_q(1, xts[1])],
                    (0, 6): [lambda: emit_chunk_q(2, xts[2])],
                    (0, 7): [lambda: emit_chunk_q(3, xts[3])],
                }
                for inst in range(1, 5):
                    ck = inst + 3
                    chunk_sched[(inst, 1)] = [
                        lambda c=ck: emit_chunk_q(c, xts[c])]
                    chunk_sched[(inst, 3)] = [
                        lambda c=ck: emit_chunk_k(c, xts[c])]
                    chunk_sched[(inst, 5)] = [
                        lambda c=ck: emit_v(c, x3view(xts[c]))]
                def emit_avs(pend_av):
                    av_, atp_, b_, kcp_ = pend_av
                    vbase = (b_ * 16 + 2 * kcp_) * 2 * VW
                    vp3 = v_big[:, vbase:vbase + 4 * VW].rearrange(
                        "p (two f) -> p two f", two=2)
                    for h in range(HPC):
                        vph = vp3[:, :, h * VW:h * VW + 65]
                        at3 = atp_[:, ts(h, 1024)].rearrange(
                            "p (u q) -> p u q", u=2)
                        nc.tensor.matmul(av_[h], vph, at3,
                                         start=(kcp_ == 0), stop=(kcp_ == 7),
                                         perf_mode=DR)

                prev = None          # (q0, av) of previous instance
                pend = None          # AV emission delayed by one kcp slot
                for inst in range(8):
                    b, qc = divmod(inst, 4)
                    q0 = b * S + qc * 512
                    av = None
                    for kcp in range(8):
                        atp = ap_.tile([128, 2048], f8, tag="atp",
                                       name=f"atp{inst}_{kcp}")
                        atp4 = atp[:, :].rearrange("p (h u q) -> p h u q",
                                                   h=2, u=2)
                        for u in range(2):
                            kc = 2 * kcp + u
                            k0 = b * S + kc * 128
                            sc = scp.tile([128, 1024], dt, tag="sc",
                                          name=f"sc{inst}_{kcp}_{u}")
                            for h in range(HPC):
                                hp0 = h * 64
                                nc.tensor.matmul(
                                    sc[:, ts(h, 512)],
                                    kT[hp0:hp0 + 64, k0:k0 + 128],
                                    qT[hp0:hp0 + 64, q0:q0 + 512],
                                    start=True, stop=True)
                            nc.scalar.activation(
                                atp4[:, :, u, :],
                                sc[:, :].rearrange("p (h q) -> p h q", h=2),
                                Exp, scale=scl_t[:, 0:1])
                        # AV of the previous kcp, emitted after this kcp's
                        # exps: the accumulation never sits between the exp
                        # stream and the next score matmuls on PE.
                        if pend is not None:
                            emit_avs(pend)
                            pend = None
                        if kcp == 0:
                            # deferred epilogue A of the previous instance
                            if prev is not None:
                                emit_epi_a(inst - 1, prev[1])
                            av = [pav.tile([65, 512], dt, tag=f"av{h}",
                                           name=f"av{h}_{inst}")
                                  for h in range(HPC)]
                        # interleaved filler work (ahead of this kcp's AV so
                        # same-kcp v blocks are produced before their reader)
                        for act in chunk_sched.get((inst, kcp), ()):
                            act()
                        pend = (av, atp, b, kcp)
                        if kcp == 2 and prev is not None:
                            emit_epi_b(inst - 1, prev[0])
                        if prev is not None and kcp in (3, 4, 6, 7):
                            emit_wo(prev[0], (3, 4, 6, 7).index(kcp))
                        if kcp == 6 and inst <= 3:
                            xts[inst + 4] = emit_xt(inst + 4)
                    prev = (q0, av)
                emit_avs(pend)
                pend = None
                # tail: last instance's epilogue + WO, evac split ACT/DVE;
                # aout writes at 128-col grain so the first WO starts early
                emit_epi_a(7, prev[1], last=True)
                q0p = prev[0]
                bcs = []
                for h in range(HPC):
                    rden, avs = epi_box[7][h]
                    bc = pav.tile([64, 512], dt, tag=f"av{h}",
                                  name=f"bct{h}")
                    nc.tensor.matmul(bc, ones_t, rden, start=True, stop=True)
                    bcs.append((avs, bc))
                for i in range(4):
                    for h in range(HPC):
                        avs, bc = bcs[h]
                        sl0 = h * N + q0p + i * 128
                        nc.vector.tensor_mul(
                            aout[0:64, sl0:sl0 + 128],
                            avs[:, i * 128:(i + 1) * 128],
                            bc[:, i * 128:(i + 1) * 128])
                for i in range(4):
                    emit_wo(q0p, i, mix=True)

    nc.compile()
    return nc


def _build_moe(cap=CAP):
    """Launch B: routed expert (cap tokens) + shared expert (SHARE tokens)."""
    import concourse.mybir as mybir
    import concourse.tile as tile
    from concourse.bass import ts

    TPAD = cap + SHARE
    dt = mybir.dt.float32
    f8 = mybir.dt.float8e4
    f16 = mybir.dt.float16
    DR = mybir.MatmulPerfMode.DoubleRow
    nc = _mk_bass()

    NTT = TPAD // 128
    xft = nc.dram_tensor("xft", [D, TPAD], f8, kind="ExternalInput").ap()
    # w1|w3 interleaved per hc block: cols hc*2048+[0:1024]=w1, [1024:2048]=w3
    w13e = nc.dram_tensor("w13e", [128, 2 * KCB * HM], f8,
                          kind="ExternalInput").ap()
    w2e = nc.dram_tensor("w2e", [128, HCB * D], f8, kind="ExternalInput").ap()
    w13s = nc.dram_tensor("w13s", [128, 2 * KCB * HM], f8,
                          kind="ExternalInput").ap()
    w2s = nc.dram_tensor("w2s", [128, HCB * D], f8, kind="ExternalInput").ap()
    # cols 0-1: silu/h13 dequants; cols 2..2+NTT: per-token-tile out scales
    sconst = nc.dram_tensor("sconst", [128, 2 + NTT], dt,
                            kind="ExternalInput").ap()
    out = nc.dram_tensor("out", [TPAD, D], f16, kind="ExternalOutput").ap()

    out_r = out.rearrange("(tt p) d -> tt p d", p=128)
    xft_rr = xft.rearrange("(kc p) t -> p kc t", p=128)

    # shared chunk FIRST (its weights stream in first) so the smallest
    # routed chunk lands last and the drain tail is short
    chunks = [(cap, SHARE, True)]
    c0 = 0
    while c0 < cap:
        w = min(512, cap - c0)
        chunks.append((c0, w, False))
        c0 += w

    with tile.TileContext(nc) as tc:
        with (
            tc.tile_pool(name="xf", bufs=1) as xfp,
            tc.tile_pool(name="wts", bufs=1) as wp,
            tc.tile_pool(name="h13", bufs=1) as hp,
            tc.tile_pool(name="scl", bufs=1) as scp,
        ):
            xfbig = xfp.tile([128, KCB * TPAD], f8, tag="xf")
            h13big = hp.tile([128, HCB * TPAD], f8, tag="h13")
            w13b = wp.tile([128, 2 * KCB * HM], f8, tag="w13b")
            w13sb = wp.tile([128, 2 * KCB * HM], f8, tag="w13sb")
            w2b = wp.tile([128, HCB * D], f8, tag="w2b")
            w2sb = wp.tile([128, HCB * D], f8, tag="w2sb")
            sca = scp.tile([128, 2 + NTT], dt, tag="sctb")
            scl_t = sca[:, 0:2]
            sct_b = sca[:, 2:2 + NTT]

            # DMA priority order: shared chunk (processed first) streams in
            # first, then routed; everything on the SP queue so the
            # exclusive DMA device serves them in this order.
            nc.sync.dma_start(out=sca, in_=sconst)
            xf3o = xfbig[:, :].rearrange("p (kc t) -> p kc t", kc=KCB)
            nc.sync.dma_start(out=w13sb[:, 0:2048], in_=w13s[:, 0:2048])
            nc.sync.dma_start(out=xf3o[:, :, cap:TPAD],
                              in_=xft_rr[:, :, cap:TPAD])
            nc.sync.dma_start(out=w13sb[:, 2048:4096], in_=w13s[:, 2048:4096])
            nc.sync.dma_start(out=w13sb[:, 4096:16384],
                              in_=w13s[:, 4096:16384])
            nc.sync.dma_start(out=xf3o[:, :, 0:cap], in_=xft_rr[:, :, 0:cap])
            nc.sync.dma_start(out=w13b[:, 0:8192], in_=w13e[:, 0:8192])
            nc.sync.dma_start(out=w2sb, in_=w2s)
            nc.sync.dma_start(out=w13b[:, 8192:16384],
                              in_=w13e[:, 8192:16384])
            nc.sync.dma_start(out=w2b, in_=w2e)

            xf3 = xfbig[:, :].rearrange("p (kc t) -> p kc t", kc=KCB)
            h13_3 = h13big[:, :].rearrange("p (hc t) -> p hc t", hc=HCB)

            with (
                tc.tile_pool(name="silu", bufs=3) as sp_,
                tc.tile_pool(name="oout", bufs=4) as op_,
                tc.tile_pool(name="ps_b1", bufs=2, space="PSUM") as ps1,
                tc.tile_pool(name="ps_b2", bufs=2, space="PSUM") as ps2,
            ):
                def emit_p2(c0, is_sh, i):
                    # phase-2 for one 128-token tile; dequant alternates
                    # DVE/ACT so neither engine backs the PE stream up
                    tt = c0 // 128 + i
                    w2T = w2sb if is_sh else w2b
                    w2_3 = w2T[:, :].rearrange("p (hc d) -> p hc d", hc=HCB)
                    po = ps2.tile([128, 1024], dt, tag="po", name=f"po{tt}")
                    for hc2 in range(HCB // 2):
                        st, sp = hc2 == 0, hc2 == HCB // 2 - 1
                        lh = h13_3[:, 2 * hc2:2 * hc2 + 2,
                                   tt * 128:(tt + 1) * 128]
                        for j in range(2):
                            nc.tensor.matmul(
                                po[:, ts(j, 512)], lh,
                                w2_3[:, 2 * hc2:2 * hc2 + 2, ts(j, 512)],
                                start=st, stop=sp, perf_mode=DR)
                    ot = op_.tile([128, 1024], f16, tag="ot", name=f"ot{tt}")
                    if tt % 2 == 0 and tt != NTT - SHARE // 128 - 1:
                        nc.vector.tensor_scalar_mul(ot, po,
                                                    sct_b[:, tt:tt + 1])
                    else:
                        nc.scalar.activation(ot, po,
                                             mybir.ActivationFunctionType.Copy,
                                             scale=sct_b[:, tt:tt + 1])
                    nc.scalar.dma_start(out=out_r[tt], in_=ot)

                pend_p2 = []
                for (c0, cw, is_sh) in chunks:
                    w13T = w13sb if is_sh else w13b
                    p2q = list(pend_p2)
                    pend_p2 = []
                    for hc in range(HCB):
                        p1 = ps1.tile([128, 512], dt, tag="p1",
                                      name=f"p1_{c0}_{hc}")
                        p3 = ps1.tile([128, 512], dt, tag="p3",
                                      name=f"p3_{c0}_{hc}")
                        for kc2 in range(KCB // 2):
                            st, sp = kc2 == 0, kc2 == KCB // 2 - 1
                            rh = xf3[:, 2 * kc2:2 * kc2 + 2, c0:c0 + cw]
                            o1 = hc * 2048 + kc2 * 256
                            o3 = hc * 2048 + 1024 + kc2 * 256
                            nc.tensor.matmul(
                                p1[:, 0:cw],
                                w13T[:, o1:o1 + 256].rearrange(
                                    "p (two m) -> p two m", two=2),
                                rh, start=st, stop=sp, perf_mode=DR)
                            nc.tensor.matmul(
                                p3[:, 0:cw],
                                w13T[:, o3:o3 + 256].rearrange(
                                    "p (two m) -> p two m", two=2),
                                rh, start=st, stop=sp, perf_mode=DR)
                        # h13 = silu(z1) * z3_psum; z3 psum carries scale
                        # SXF*SW3 = 16 so the fp8 h13 stays in range and the
                        # dequant folds into the per-token output scale.
                        silu8 = sp_.tile([128, 512], f8, tag="silu",
                                         name=f"si_{c0}_{hc}")
                        nc.scalar.activation(silu8[:, 0:cw], p1[:, 0:cw],
                                             mybir.ActivationFunctionType.Silu,
                                             scale=scl_t[:, 0:1])
                        nc.vector.tensor_mul(
                            h13big[:, hc * TPAD + c0:hc * TPAD + c0 + cw],
                            silu8[:, 0:cw], p3[:, 0:cw])
                        # previous chunk's phase 2, pipelined into this one
                        if hc % 2 == 1 and p2q:
                            p2q.pop(0)()
                    for fn in p2q:
                        fn()
                    pend_p2 = [
                        (lambda c=c0, s=is_sh, i=i: emit_p2(c, s, i))
                        for i in range(cw // 128)]
                for fn in pend_p2:
                    fn()

    nc.compile()
    return nc


def _programs():
    if "A" not in _CACHE:
        _CACHE["A"] = _build_attn()
    if "Bp" not in _CACHE:
        _CACHE["Bp"] = _build_moe()
    return _CACHE["A"], _CACHE["Bp"]


def _run(nc, in_maps, trace=False):
    from concourse.bass_utils import run_bass_kernel_spmd
    return run_bass_kernel_spmd(nc, in_maps, list(range(NCORES)), trace=trace)


# --------------------------------------------------------------------------
# host-side orchestration
# --------------------------------------------------------------------------

def _rmsnorm(x, w):
    return x * (1.0 / np.sqrt((x * x).mean(-1, keepdims=True) + EPS)) * w


# head-contiguous layout: head h occupies partitions h*64:(h+1)*64, with
# evens in the first 32 rows and odds in the next 32 — no score masks needed.
_PERM = np.concatenate([
    np.arange(0, 64, 2), np.arange(1, 64, 2),
    64 + np.arange(0, 64, 2), 64 + np.arange(1, 64, 2),
])
_PERMS = np.concatenate([
    np.arange(1, 64, 2), np.arange(0, 64, 2),
    64 + np.arange(1, 64, 2), 64 + np.arange(0, 64, 2),
])  # even<->odd partner swap per head


def _pack_kc(w):
    # (1024, 128) -> (128, 8*128): block kc holds rows kc*128..kc*128+127
    return np.ascontiguousarray(
        w.reshape(8, 128, 128).transpose(1, 0, 2).reshape(128, 1024))


def prep_attn_inputs(x, freqs_cos, freqs_sin, att_norm_w, wq, wk, wv, wo):
    xn = _rmsnorm(x.reshape(N, D), att_norm_w)
    sx = _p2(64.0, np.abs(xn).max())
    wk_s = wk * (1.0 / np.sqrt(HD))
    swq = _p2(64.0, np.abs(wq).max())
    swk = _p2(64.0, np.abs(wk_s).max())
    swv = _p2(64.0, np.abs(wv).max())
    swo = _p2(64.0, np.abs(wo).max())
    # rope output scale folding: cos tile carries SQ8/(sx*swq); k reuses the
    # same tile so its fp8 out scale is sk8 = swk*SQ8/swq.
    sk8 = swk * SQ8 / swq
    cscale = SQ8 / (sx * swq)

    xnt8 = np.ascontiguousarray(xn.T * sx).astype(E4NP)
    cosT = np.ascontiguousarray(freqs_cos.T)    # (32, S)
    sinT = np.ascontiguousarray(freqs_sin.T)
    cos2 = (np.tile(cosT, (4, 1)) * cscale).astype(BF16NP)    # (128, S)
    sin2 = np.tile(sinT, (4, 1)) * cscale
    sin2[32:64, :] *= -1.0    # odd-dim rows (o1 = t1*c + t0*s)
    sin2[96:128, :] *= -1.0
    sin2n = np.ascontiguousarray(sin2).astype(BF16NP)

    sconst = np.zeros((128, 3), FP32)
    sconst[:, 0] = 1.0 / (SQ8 * sk8)          # exp dequant
    sconst[:, 1] = SV8 / (sx * swv)           # v requant
    sconst[:, 2] = 1.0 / (SV8 * swo)          # partial dequant

    cspack = np.ascontiguousarray(np.hstack([cos2, sin2n]))   # (128, 2S)
    xk = xnt8.reshape(8, 128, N)
    x0a = xk[0:4, :, 0:512].transpose(1, 0, 2).reshape(128, 2048)
    x0b = xk[4:8, :, 0:512].transpose(1, 0, 2).reshape(128, 2048)
    in_maps = []
    for c in range(NCORES):
        blk = slice(c * 128, (c + 1) * 128)
        wo_blk = wo[blk, :]
        wo2 = np.concatenate([wo_blk[0:64, :], wo_blk[64:128, :]],
                             axis=1) * swo   # (64, 2048)
        spk = np.hstack([
            x0a,
            _pack_kc(wq[:, blk][:, _PERM] * swq).astype(E4NP),
            _pack_kc(wq[:, blk][:, _PERMS] * swq).astype(E4NP),
            x0b,
            _pack_kc(wk_s[:, blk][:, _PERM] * swk).astype(E4NP),
            _pack_kc(wk_s[:, blk][:, _PERMS] * swk).astype(E4NP),
            _pack_kc(wv[:, blk] * swv).astype(E4NP),
        ])
        in_maps.append({
            "xnt": xnt8,
            "spk": np.ascontiguousarray(spk),
            "wo2": np.ascontiguousarray(wo2).astype(E4NP),
            "cspack": cspack,
            "onesv": np.ones((1, 64), FP32),
            "sconst": sconst,
        })
    return in_maps


def route(xf, gate_w):
    g = xf @ gate_w.T
    g = g - g.max(-1, keepdims=True)
    p = np.exp(g)
    p /= p.sum(-1, keepdims=True)
    idx = np.argsort(-p, axis=1, kind="stable")[:, :TOPK]      # (N, 2)
    vals = np.take_along_axis(p, idx, axis=1)
    w = vals / (vals.sum(-1, keepdims=True) + 1e-9)
    experts = []
    for e in range(E):
        m = idx == e
        tok = np.nonzero(m.any(1))[0]
        wt = (w * m).sum(1)[tok]
        experts.append((tok, wt.astype(FP32)))
    return experts


def _pack_w13(w, s):
    # (D, HM) -> (128, 8*1024): col = hc*1024 + kc*128 + m  (hc-major)
    return np.ascontiguousarray(
        (w * s).reshape(8, 128, 8, 128).transpose(1, 2, 0, 3).reshape(128, 8192)
    ).astype(E4NP)


def _pack_w2(w, s):
    # (HM, D) -> (128, 8*1024): col = hc*1024 + d
    return np.ascontiguousarray(
        (w * s).reshape(8, 128, D).transpose(1, 0, 2).reshape(128, 8192)
    ).astype(E4NP)


def _pack_w13x(w1, w3, s1, s3):
    # interleave the two hc-major packs per 1024-col hc block
    a = _pack_w13(w1, s1).reshape(128, 8, 1024)
    b = _pack_w13(w3, s3).reshape(128, 8, 1024)
    return np.ascontiguousarray(
        np.concatenate([a, b], axis=2).reshape(128, 16384))


def kernel(**inputs):
    ins = {k: np.ascontiguousarray(np.asarray(v)) for k, v in inputs.items()}
    x = ins["x"].astype(FP32, copy=False)
    nc_a, _ = _programs()

    # ----- launch A: attention -----
    in_maps = prep_attn_inputs(
        x, ins["freqs_cos"], ins["freqs_sin"], ins["att_norm_w"],
        ins["wq"], ins["wk"], ins["wv"], ins["wo"],
    )
    res_a = _run(nc_a, in_maps, trace=_CACHE.get("trace", False))
    _CACHE["res_a"] = res_a

    h = x.reshape(N, D).copy()
    for c in range(NCORES):
        h += res_a.results[c]["partial"].astype(FP32)

    # ----- host routing -----
    xf = _rmsnorm(h, ins["ffn_norm_w"])
    experts = route(xf, ins["gate_w"])

    max_ct = max(len(t) for t, _ in experts)
    cap = CAP if max_ct <= CAP else ((max_ct + 127) // 128) * 128
    key = f"Bp{cap}"
    if key not in _CACHE:
        _CACHE[key] = _CACHE.get("Bp") if cap == CAP else _build_moe(cap)
        if _CACHE[key] is None:
            _CACHE[key] = _build_moe(cap)
    nc_b = _CACHE[key]
    tpad = cap + SHARE

    sw1 = _p2(64.0, max(np.abs(ins["ew1"]).max(), np.abs(ins["sw1"]).max()))
    sw3 = SW3
    sw2 = _p2(64.0, max(np.abs(ins["ew2"]).max(), np.abs(ins["sw2"]).max()))
    ntt = tpad // 128
    sconst0 = np.zeros((128, 2), FP32)
    sconst0[:, 0] = 1.0 / (SXF * sw1)
    sconst0[:, 1] = 1.0 / (SXF * sw3)

    xf8 = (xf * SXF).astype(E4NP)     # (N, D)
    w13s_p = _pack_w13x(ins["sw1"], ins["sw3"], sw1, sw3)
    w2s_p = _pack_w2(ins["sw2"], sw2)

    in_maps_b = []
    for c in range(NCORES):
        tok, wt = experts[c]
        ct = len(tok)
        xft = np.zeros((D, tpad), E4NP)
        xft[:, :ct] = xf8[tok].T
        xft[:, cap:] = xf8[c * SHARE:(c + 1) * SHARE].T
        sc = np.zeros((tpad,), FP32)
        sc[:ct] = wt / (SXF * sw3 * sw2)
        sc[cap:] = 1.0 / (SXF * sw3 * sw2)
        sconst = np.hstack([sconst0, sc.reshape(ntt, 128).T]).astype(FP32)
        in_maps_b.append({
            "xft": xft,
            "w13e": _pack_w13x(ins["ew1"][c], ins["ew3"][c], sw1, sw3),
            "w2e": _pack_w2(ins["ew2"][c], sw2),
            "w13s": w13s_p,
            "w2s": w2s_p,
            "sconst": np.ascontiguousarray(sconst),
        })
    res_b = _run(nc_b, in_maps_b, trace=_CACHE.get("trace", False))
    _CACHE["res_b"] = res_b

    # ----- combine -----
    y = h.copy()
    for c in range(NCORES):
        o = res_b.results[c]["out"].astype(FP32)
        tok, _ = experts[c]
        ct = len(tok)
        y[tok] += o[:ct]
        y[c * SHARE:(c + 1) * SHARE] += o[cap:]
    return y.reshape(B, S, D).astype(ins["x"].dtype, copy=False)


# revision 58
# speedup vs baseline: 1.0035x; 1.0035x over previous
"""Trainium2 Bass kernel for nn_Block (attention + MoE routing block), fp8.

Contract: kernel(**inputs) takes FULL unsharded inputs, returns FULL output.
Internally: two SPMD launches over 8 NeuronCores.
  Launch A: attention, tensor-parallel over heads (2 heads per core).
            Single interleaved stream: the softmax exp on ACT is the serial
            floor (~133us); QKV+rope, scores, AV, WO are all scheduled to
            hide under it (rolling start on batch 0, deferred epilogue/WO).
  Launch B: MoE, expert-parallel (1 routed expert per core) + data-parallel
            shared expert; host routing between launches. fp8 DoubleRow for
            all three matmul stages, ACT Silu.
"""

import numpy as np
import ml_dtypes

E4NP = ml_dtypes.float8_e4m3
BF16NP = ml_dtypes.bfloat16

# ---- problem shapes (hardcoded per contract) ----
B, S, D, H, HD = 2, 2048, 1024, 16, 64
E, TOPK = 8, 2
HM = 1024   # moe inter dim
HS = 1024   # shared expert hidden
N = B * S   # 4096 tokens
NCORES = 8
HPC = H // NCORES          # heads per core = 2
CAP = 1152                 # routed-token capacity per expert
SHARE = N // NCORES        # shared-expert tokens per core = 512
EPS = 1e-6
FP32 = np.float32
KCB = D // 128   # 8 contraction blocks
HCB = HM // 128  # 8 inter blocks
VW = 80          # per-head v block width (64 dims + 1 ones + pad)

# fp8 scale plan (pow2). Device-value scales fixed; weight scales computed
# per call on host, dequants ride in via input APs.
SQ8 = 32.0                 # q rope-out fp8 scale
SV8 = 32.0                 # v fp8 scale
SXF = 2.0                  # moe input fp8 scale
SW3 = 8.0                  # moe w3 fp8 scale; SXF*SW3=16 keeps h13 in fp8 range

_CACHE = {}


def _p2(target, amax):
    if amax <= 0:
        return 1.0
    return float(2.0 ** np.floor(np.log2(target / amax)))


# --------------------------------------------------------------------------
# device programs
# --------------------------------------------------------------------------

def _mk_bass():
    from concourse import bacc
    return bacc.Bacc(
        "TRN2",
        target_bir_lowering=False,
        debug=False,
        enable_asserts=True,
        num_devices=NCORES,
    )


def _build_attn():
    """Launch A: per-core attention for 2 heads; outputs partial (N, D) of @wo.

    One interleaved stream.  ACT runs only the 128 softmax exps ([128,1024]
    each, ~133us total) -- everything else hides under them:
      - prologue: QKV+rope for token chunks 0..2 (batch 0 mostly)
      - 8 instances of (batch, 512-query chunk): per kcp (256 keys) 2 score
        psums -> 2 exps -> 2 AV DoubleRow accums
      - chunks 3..7 QKV+rope interleaved into instance kcp slots
      - softmax epilogue + WO of instance i deferred into instance i+1
    """
    import concourse.mybir as mybir
    import concourse.tile as tile
    from concourse.bass import ts

    dt = mybir.dt.float32
    dtr = mybir.dt.float32r
    f8 = mybir.dt.float8e4
    bf = mybir.dt.bfloat16
    f16 = mybir.dt.float16
    DR = mybir.MatmulPerfMode.DoubleRow
    Exp = mybir.ActivationFunctionType.Exp
    Copy = mybir.ActivationFunctionType.Copy
    nc = _mk_bass()

    xnt = nc.dram_tensor("xnt", [D, N], f8, kind="ExternalInput").ap()
    # startup pack: [x0(kc0-3) | wq | wqs | x0(kc4-7) | wk | wks | wv] so the
    # first-chunk critical chain is two DMAs
    spk = nc.dram_tensor("spk", [128, 9216], f8, kind="ExternalInput").ap()
    wo2 = nc.dram_tensor("wo2", [64, 2048], f8, kind="ExternalInput").ap()
    # cos|sin packed -> two DMAs (first 512 cols, then the rest)
    cspack = nc.dram_tensor("cspack", [128, 2 * S], bf,
                            kind="ExternalInput").ap()
    onesv = nc.dram_tensor("onesv", [1, 64], dtr, kind="ExternalInput").ap()
    # sconst cols: 0=exp scale, 1=v requant, 2=partial dequant
    sconst = nc.dram_tensor("sconst", [128, 3], dt, kind="ExternalInput").ap()
    partial = nc.dram_tensor("partial", [N, D], f16, kind="ExternalOutput").ap()

    xnt_r = xnt.rearrange("(kc p) t -> kc p t", p=128)        # 8 x 128 x 4096
    partial_r2 = partial.rearrange("(g two p) d -> g p two d", p=128, two=2)

    KC = D // 128      # 8 contraction chunks

    with tile.TileContext(nc) as tc:
        with tc.tile_pool(name="persist", bufs=1) as pp:
            qT = pp.tile([128, N], f8, tag="qT")
            kT = pp.tile([128, N], f8, tag="kT")
            cs_b = pp.tile([128, 2 * S], bf, tag="cs")
            cos_t = cs_b[:, 0:S]
            sin_t = cs_b[:, S:2 * S]
            aout = pp.tile([64, 2 * N], f8, tag="aout")     # [h0 | h1] blocks
            ones_t = pp.tile([1, 64], dtr, tag="ones")
            scl_t = pp.tile([128, 3], dt, tag="scl")
            swall = pp.tile([128, 9216], f8, tag="swall")
            wq_b = swall[:, 2048:3072]
            wqs_b = swall[:, 3072:4096]
            wk_b = swall[:, 6144:7168]
            wks_b = swall[:, 7168:8192]
            wv_b = swall[:, 8192:9216]
            wo_b = pp.tile([64, 2048], f8, tag="wob")
            v_big = pp.tile([128, 32 * 2 * VW], f8, tag="vbig")

            def w3v(t):
                return [t[:, kc2 * 256:(kc2 + 1) * 256].rearrange(
                    "p (two m) -> p two m", two=2) for kc2 in range(KC // 2)]

            wq3, wqs3 = w3v(wq_b), w3v(wqs_b)
            wk3, wks3 = w3v(wk_b), w3v(wks_b)
            wv3 = w3v(wv_b)
            wo3 = wo_b[:, :].rearrange("p (two d) -> p two d", two=2)
            aout3 = aout[:, :].rearrange("p (two t) -> p two t", two=2)

            with (
                tc.tile_pool(name="xnstream", bufs=4) as xp,
                tc.tile_pool(name="ropetmp", bufs=4) as rp,
                tc.tile_pool(name="attn", bufs=2) as ap_,
                tc.tile_pool(name="oout", bufs=4) as op_,
                tc.tile_pool(name="norm", bufs=2) as np_,
                tc.tile_pool(name="ps_sc", bufs=2, space="PSUM") as scp,
                tc.tile_pool(name="ps_av", bufs=1, space="PSUM") as pav,
                tc.tile_pool(name="ps_misc", bufs=2, space="PSUM") as p3,
            ):
                # ---------- emission helpers ----------
                def emit_xt(tcix):
                    xt = xp.tile([128, 4096], f8, tag="xn", name=f"xn{tcix}")
                    nc.sync.dma_start(
                        out=xt[:, :].rearrange("p (kc t) -> p kc t", kc=KC),
                        in_=xnt_r.transpose([1, 0, 2])[:, :, ts(tcix, 512)])
                    return xt

                def x3view(xt):
                    return [xt[:, kc2 * 1024:(kc2 + 1) * 1024].rearrange(
                        "p (two t) -> p two t", two=2) for kc2 in range(KC // 2)]

                def rope_out(pc_lo, pc_hi, dstT, tcix, halves=1):
                    c0 = (tcix * 512) % S
                    hw = 512 // halves
                    for hf in range(halves):
                        o = hf * hw
                        a = rp.tile([128, hw], dtr, tag="ra",
                                    name=f"ra{tcix}_{id(pc_lo)%97}_{hf}")
                        bb = rp.tile([128, hw], dtr, tag="rb",
                                     name=f"rb{tcix}_{id(pc_lo)%97}_{hf}")
                        nc.vector.tensor_mul(a, pc_lo[:, o:o + hw],
                                             cos_t[:, c0 + o:c0 + o + hw])
                        nc.vector.tensor_mul(bb, pc_hi[:, o:o + hw],
                                             sin_t[:, c0 + o:c0 + o + hw])
                        dsl = dstT[:, tcix * 512 + o:tcix * 512 + o + hw]
                        if tcix <= 1:
                            # startup chain: DVE sub is lower-latency
                            nc.vector.tensor_sub(dsl, a, bb)
                        else:
                            nc.gpsimd.tensor_sub(dsl, a, bb)

                def emit_v(tcix, x3s):
                    pv = p3.tile([128, 512], dt, tag="pb", name=f"pv{tcix}")
                    for i in range(4):
                        for kc2 in range(KC // 2):
                            nc.tensor.matmul(
                                pv[:, ts(i, 128)],
                                x3s[kc2][:, :, ts(i, 128)], wv3[kc2],
                                start=(kc2 == 0), stop=(kc2 == KC // 2 - 1),
                                perf_mode=DR)
                    for i in range(4):
                        tt = tcix * 4 + i
                        base = tt * 2 * VW
                        dst3 = v_big[:, base:base + 2 * VW].rearrange(
                            "p (two f) -> p two f", two=2)[:, :, 0:64]
                        src3 = pv[:, ts(i, 128)].rearrange(
                            "p (two f) -> p two f", two=2)
                        nc.vector.tensor_scalar_mul(dst3, src3, scl_t[:, 1:2])

                def emit_chunk_big(tcix, x3s):
                    # prologue variant: q|qs and k|ks in [128,1024] sc-pool
                    pqc = scp.tile([128, 1024], dt, tag="sc", name=f"pqc{tcix}")
                    for kc2 in range(KC // 2):
                        st, sp = kc2 == 0, kc2 == KC // 2 - 1
                        nc.tensor.matmul(pqc[:, 0:512], wq3[kc2], x3s[kc2],
                                         start=st, stop=sp, perf_mode=DR)
                        nc.tensor.matmul(pqc[:, 512:1024], wqs3[kc2], x3s[kc2],
                                         start=st, stop=sp, perf_mode=DR)
                    rope_out(pqc[:, 0:512], pqc[:, 512:1024], qT, tcix)
                    pkc = scp.tile([128, 1024], dt, tag="sc", name=f"pkc{tcix}")
                    for kc2 in range(KC // 2):
                        st, sp = kc2 == 0, kc2 == KC // 2 - 1
                        nc.tensor.matmul(pkc[:, 0:512], wk3[kc2], x3s[kc2],
                                         start=st, stop=sp, perf_mode=DR)
                        nc.tensor.matmul(pkc[:, 512:1024], wks3[kc2], x3s[kc2],
                                         start=st, stop=sp, perf_mode=DR)
                    # chunk 0: produce kT in 256-token halves so the first
                    # score matmul starts one DVE-mul earlier
                    rope_out(pkc[:, 0:512], pkc[:, 512:1024], kT, tcix,
                             halves=2 if tcix == 0 else 1)
                    emit_v(tcix, x3s)

                def emit_chunk_q(tcix, xt):
                    x3s = x3view(xt)
                    pq = p3.tile([128, 512], dt, tag="pb", name=f"pq{tcix}")
                    for kc2 in range(KC // 2):
                        nc.tensor.matmul(pq, wq3[kc2], x3s[kc2],
                                         start=(kc2 == 0),
                                         stop=(kc2 == KC // 2 - 1), perf_mode=DR)
                    pqs = p3.tile([128, 512], dt, tag="pb", name=f"pqs{tcix}")
                    for kc2 in range(KC // 2):
                        nc.tensor.matmul(pqs, wqs3[kc2], x3s[kc2],
                                         start=(kc2 == 0),
                                         stop=(kc2 == KC // 2 - 1), perf_mode=DR)
                    rope_out(pq, pqs, qT, tcix)

                def emit_chunk_k(tcix, xt):
                    x3s = x3view(xt)
                    pk = p3.tile([128, 512], dt, tag="pb", name=f"pk{tcix}")
                    for kc2 in range(KC // 2):
                        nc.tensor.matmul(pk, wk3[kc2], x3s[kc2],
                                         start=(kc2 == 0),
                                         stop=(kc2 == KC // 2 - 1), perf_mode=DR)
                    pks = p3.tile([128, 512], dt, tag="pb", name=f"pks{tcix}")
                    for kc2 in range(KC // 2):
                        nc.tensor.matmul(pks, wks3[kc2], x3s[kc2],
                                         start=(kc2 == 0),
                                         stop=(kc2 == KC // 2 - 1), perf_mode=DR)
                    rope_out(pk, pks, kT, tcix)

                ot_box = {}

                def emit_wo(q0p, i, mix=False):
                    tt = q0p // 128 + i
                    ot = op_.tile([128, 1024], f16, tag="ot", name=f"ot{tt}")
                    lh3 = aout3[:, :, tt * 128:(tt + 1) * 128]
                    for j in range(2):
                        if mix and i >= 2:
                            # tail: av psum slots are free; 4 po tiles in
                            # flight lets PE/DVE/ACT evac run in parallel
                            po = pav.tile([128, 512], dt, tag=f"av{j}",
                                          name=f"po{tt}_{j}")
                        else:
                            po = p3.tile([128, 512], dt, tag="pb",
                                         name=f"po{tt}_{j}")
                        nc.tensor.matmul(po, lh3, wo3[:, :, ts(j, 512)],
                                         start=True, stop=True, perf_mode=DR)
                        osl = ot[:, j * 512:j * 512 + 512]
                        if mix and i >= 2:
                            nc.scalar.activation(osl, po, Copy,
                                                 scale=scl_t[:, 2:3])
                        else:
                            nc.vector.tensor_scalar_mul(osl, po,
                                                        scl_t[:, 2:3])
                    nc.sync.dma_start(out=partial_r2[tt // 2][:, tt % 2, :],
                                      in_=ot)

                epi_box = {}

                def emit_epi_a(inst, av, last=False):
                    # denominator reciprocal + numerator copy (DVE; ACT when
                    # the exp stream is over).  In the tail, interleave so
                    # ACT and DVE touch different av banks concurrently.
                    rden, avs = {}, {}
                    for h in range(HPC):
                        rden[h] = np_.tile([1, 512], dtr, tag=f"rden{h}",
                                           name=f"rden{h}_{inst}")
                        avs[h] = np_.tile([64, 512], dtr, tag=f"avs{h}",
                                          name=f"avs{h}_{inst}")
                    if last:
                        with nc.allow_low_precision("softmax denom recip"):
                            nc.vector.reciprocal(rden[0], av[0][64:65, :])
                            nc.scalar.activation(avs[1], av[1][0:64, :], Copy)
                            nc.vector.reciprocal(rden[1], av[1][64:65, :])
                            nc.scalar.activation(avs[0], av[0][0:64, :], Copy)
                    else:
                        for h in range(HPC):
                            with nc.allow_low_precision("softmax denom recip"):
                                nc.vector.reciprocal(rden[h], av[h][64:65, :])
                            nc.vector.tensor_copy(avs[h], av[h][0:64, :])
                    epi_box[inst] = [(rden[h], avs[h]) for h in range(HPC)]

                def emit_epi_b(inst, q0):
                    # broadcast recip via ones-matmul + aout write; bc lives
                    # in the misc psum pool so a late DVE recip never blocks
                    # the PE score stream (emitted 2 kcps after part A).
                    for h in range(HPC):
                        rden, avs = epi_box[inst][h]
                        bc = p3.tile([128, 512], dt, tag="pb",
                                     name=f"bc{h}_{inst}")
                        nc.tensor.matmul(bc[0:64, :], ones_t, rden,
                                         start=True, stop=True)
                        nc.vector.tensor_mul(
                            aout[0:64, h * N + q0: h * N + q0 + 512], avs,
                            bc[0:64, :])

                # ---------- initial DMAs (priority order on SP queue) ----------
                # chunk-0 critical chain first: xt0 low half, q weights,
                # first 512 cols of cos/sin, then the rest.
                xts = {}
                cs2 = cs_b[:, :].rearrange("p (two s) -> p two s", two=2)
                csp2 = cspack.rearrange("p (two s) -> p two s", two=2)
                nc.sync.dma_start(out=ones_t, in_=onesv)
                nc.sync.dma_start(out=swall[:, 0:4096], in_=spk[:, 0:4096])
                nc.sync.dma_start(out=swall[:, 4096:6144],
                                  in_=spk[:, 4096:6144])
                nc.sync.dma_start(out=cs2[:, :, 0:512], in_=csp2[:, :, 0:512])
                nc.sync.dma_start(out=swall[:, 6144:8192],
                                  in_=spk[:, 6144:8192])
                nc.sync.dma_start(out=swall[:, 8192:9216],
                                  in_=spk[:, 8192:9216])
                nc.sync.dma_start(out=scl_t, in_=sconst)
                nc.sync.dma_start(out=cs2[:, :, 512:S], in_=csp2[:, :, 512:S])
                xts[1] = emit_xt(1)
                nc.sync.dma_start(out=wo_b, in_=wo2)
                xts[2] = emit_xt(2)
                # chunk-0 x data lives inside the startup pack
                x3s0 = [swall[:, 0:2048][:, kc2 * 1024:(kc2 + 1) * 1024]
                        .rearrange("p (two t) -> p two t", two=2)
                        for kc2 in range(2)] + [
                        swall[:, 4096:6144][:, kc2 * 1024:(kc2 + 1) * 1024]
                        .rearrange("p (two t) -> p two t", two=2)
                        for kc2 in range(2)]
                # PE p-state warm-up: harmless dummy matmuls while the input
                # DMAs stream in, so chunk-0 projections run at full clock
                for wi in range(12):
                    pw = p3.tile([128, 512], dt, tag="pb", name=f"warm{wi}")
                    nc.tensor.matmul(pw[0:64, 0:64], ones_t, ones_t,
                                     start=True, stop=True)
                nc.gpsimd.memset(v_big[:, 64::VW], 1.0)
                # preload the Exp activation table while ACT is idle
                warm = np_.tile([1, 3], dtr, tag="warm")
                nc.scalar.activation(warm, scl_t[0:1, 0:3], Exp)

                # ---------- prologue: chunk 0 only ----------
                emit_chunk_big(0, x3s0)
                xts[3] = emit_xt(3)

                # ---------- instances ----------
                # instance i handles (b, qc) = divmod(i, 4).
                # inst0 absorbs chunks 1..3 (split q/k/v); inst1..4 get 4..7.
                # only K and V of chunks 1..3 gate instance 0; Q projections
                # are deferred to late kcp slots (first needed by inst 1).
                chunk_sched = {
                    (0, 0): [lambda: emit_chunk_k(1, xts[1])],
                    (0, 1): [lambda: emit_v(1, x3view(xts[1])),
                             lambda: emit_chunk_k(2, xts[2])],
                    (0, 2): [lambda: emit_v(2, x3view(xts[2]))],
                    (0, 3): [lambda: emit_chunk_k(3, xts[3])],
                    (0, 4): [lambda: emit_v(3, x3view(xts[3]))],
                    (0, 5): [lambda: emit_chunk_q(1, xts[1])],
                    (0, 6): [lambda: emit_chunk_q(2, xts[2])],
                    (0, 7): [lambda: emit_chunk_q(3, xts[3])],
                }
                for inst in range(1, 5):
                    ck = inst + 3
                    chunk_sched[(inst, 1)] = [
                        lambda c=ck: emit_chunk_q(c, xts[c])]
                    chunk_sched[(inst, 3)] = [
                        lambda c=ck: emit_chunk_k(c, xts[c])]
                    chunk_sched[(inst, 5)] = [
                        lambda c=ck: emit_v(c, x3view(xts[c]))]
                def emit_avs(pend_av):
                    av_, atp_, b_, kcp_ = pend_av
                    vbase = (b_ * 16 + 2 * kcp_) * 2 * VW
                    vp3 = v_big[:, vbase:vbase + 4 * VW].rearrange(
                        "p (two f) -> p two f", two=2)
                    for h in range(HPC):
                        vph = vp3[:, :, h * VW:h * VW + 65]
                        at3 = atp_[:, ts(h, 1024)].rearrange(
                            "p (u q) -> p u q", u=2)
                        nc.tensor.matmul(av_[h], vph, at3,
                                         start=(kcp_ == 0), stop=(kcp_ == 7),
                                         perf_mode=DR)

                prev = None          # (q0, av) of previous instance
                pend = None          # AV emission delayed by one kcp slot
                for inst in range(8):
                    b, qc = divmod(inst, 4)
                    q0 = b * S + qc * 512
                    av = None
                    for kcp in range(8):
                        atp = ap_.tile([128, 2048], f8, tag="atp",
                                       name=f"atp{inst}_{kcp}")
                        atp4 = atp[:, :].rearrange("p (h u q) -> p h u q",
                                                   h=2, u=2)
                        for u in range(2):
                            kc = 2 * kcp + u
                            k0 = b * S + kc * 128
                            sc = scp.tile([128, 1024], dt, tag="sc",
                                          name=f"sc{inst}_{kcp}_{u}")
                            for h in range(HPC):
                                hp0 = h * 64
                                nc.tensor.matmul(
                                    sc[:, ts(h, 512)],
                                    kT[hp0:hp0 + 64, k0:k0 + 128],
                                    qT[hp0:hp0 + 64, q0:q0 + 512],
                                    start=True, stop=True)
                            nc.scalar.activation(
                                atp4[:, :, u, :],
                                sc[:, :].rearrange("p (h q) -> p h q", h=2),
                                Exp, scale=scl_t[:, 0:1])
                        # AV of the previous kcp, emitted after this kcp's
                        # exps: the accumulation never sits between the exp
                        # stream and the next score matmuls on PE.
                        if pend is not None:
                            emit_avs(pend)
                            pend = None
                        if kcp == 0:
                            # deferred epilogue A of the previous instance
                            if prev is not None:
                                emit_epi_a(inst - 1, prev[1])
                            av = [pav.tile([65, 512], dt, tag=f"av{h}",
                                           name=f"av{h}_{inst}")
                                  for h in range(HPC)]
                        # interleaved filler work (ahead of this kcp's AV so
                        # same-kcp v blocks are produced before their reader)
                        for act in chunk_sched.get((inst, kcp), ()):
                            act()
                        pend = (av, atp, b, kcp)
                        if kcp == 2 and prev is not None:
                            emit_epi_b(inst - 1, prev[0])
                        if prev is not None and kcp in (3, 4, 6, 7):
                            emit_wo(prev[0], (3, 4, 6, 7).index(kcp))
                        if kcp == 6 and inst <= 3:
                            xts[inst + 4] = emit_xt(inst + 4)
                    prev = (q0, av)
                emit_avs(pend)
                pend = None
                # tail: last instance's epilogue + WO, evac split ACT/DVE
                emit_epi_a(7, prev[1], last=True)
                emit_epi_b(7, prev[0])
                for i in range(4):
                    emit_wo(prev[0], i, mix=True)

    nc.compile()
    return nc


def _build_moe(cap=CAP):
    """Launch B: routed expert (cap tokens) + shared expert (SHARE tokens)."""
    import concourse.mybir as mybir
    import concourse.tile as tile
    from concourse.bass import ts

    TPAD = cap + SHARE
    dt = mybir.dt.float32
    f8 = mybir.dt.float8e4
    f16 = mybir.dt.float16
    DR = mybir.MatmulPerfMode.DoubleRow
    nc = _mk_bass()

    NTT = TPAD // 128
    xft = nc.dram_tensor("xft", [D, TPAD], f8, kind="ExternalInput").ap()
    # w1|w3 interleaved per hc block: cols hc*2048+[0:1024]=w1, [1024:2048]=w3
    w13e = nc.dram_tensor("w13e", [128, 2 * KCB * HM], f8,
                          kind="ExternalInput").ap()
    w2e = nc.dram_tensor("w2e", [128, HCB * D], f8, kind="ExternalInput").ap()
    w13s = nc.dram_tensor("w13s", [128, 2 * KCB * HM], f8,
                          kind="ExternalInput").ap()
    w2s = nc.dram_tensor("w2s", [128, HCB * D], f8, kind="ExternalInput").ap()
    # cols 0-1: silu/h13 dequants; cols 2..2+NTT: per-token-tile out scales
    sconst = nc.dram_tensor("sconst", [128, 2 + NTT], dt,
                            kind="ExternalInput").ap()
    out = nc.dram_tensor("out", [TPAD, D], f16, kind="ExternalOutput").ap()

    out_r = out.rearrange("(tt p) d -> tt p d", p=128)
    xft_rr = xft.rearrange("(kc p) t -> p kc t", p=128)

    # shared chunk FIRST (its weights stream in first) so the smallest
    # routed chunk lands last and the drain tail is short
    chunks = [(cap, SHARE, True)]
    c0 = 0
    while c0 < cap:
        w = min(512, cap - c0)
        chunks.append((c0, w, False))
        c0 += w

    with tile.TileContext(nc) as tc:
        with (
            tc.tile_pool(name="xf", bufs=1) as xfp,
            tc.tile_pool(name="wts", bufs=1) as wp,
            tc.tile_pool(name="h13", bufs=1) as hp,
            tc.tile_pool(name="scl", bufs=1) as scp,
        ):
            xfbig = xfp.tile([128, KCB * TPAD], f8, tag="xf")
            h13big = hp.tile([128, HCB * TPAD], f8, tag="h13")
            w13b = wp.tile([128, 2 * KCB * HM], f8, tag="w13b")
            w13sb = wp.tile([128, 2 * KCB * HM], f8, tag="w13sb")
            w2b = wp.tile([128, HCB * D], f8, tag="w2b")
            w2sb = wp.tile([128, HCB * D], f8, tag="w2sb")
            sca = scp.tile([128, 2 + NTT], dt, tag="sctb")
            scl_t = sca[:, 0:2]
            sct_b = sca[:, 2:2 + NTT]

            # DMA priority order: shared chunk (processed first) streams in
            # first, then routed; everything on the SP queue so the
            # exclusive DMA device serves them in this order.
            nc.sync.dma_start(out=sca, in_=sconst)
            xf3o = xfbig[:, :].rearrange("p (kc t) -> p kc t", kc=KCB)
            nc.sync.dma_start(out=w13sb[:, 0:2048], in_=w13s[:, 0:2048])
            nc.sync.dma_start(out=xf3o[:, :, cap:TPAD],
                              in_=xft_rr[:, :, cap:TPAD])
            nc.sync.dma_start(out=w13sb[:, 2048:4096], in_=w13s[:, 2048:4096])
            nc.sync.dma_start(out=w13sb[:, 4096:16384],
                              in_=w13s[:, 4096:16384])
            nc.sync.dma_start(out=xf3o[:, :, 0:cap], in_=xft_rr[:, :, 0:cap])
            nc.sync.dma_start(out=w13b[:, 0:8192], in_=w13e[:, 0:8192])
            nc.sync.dma_start(out=w2sb, in_=w2s)
            nc.sync.dma_start(out=w13b[:, 8192:16384],
                              in_=w13e[:, 8192:16384])
            nc.sync.dma_start(out=w2b, in_=w2e)

            xf3 = xfbig[:, :].rearrange("p (kc t) -> p kc t", kc=KCB)
            h13_3 = h13big[:, :].rearrange("p (hc t) -> p hc t", hc=HCB)

            with (
                tc.tile_pool(name="silu", bufs=3) as sp_,
                tc.tile_pool(name="oout", bufs=4) as op_,
                tc.tile_pool(name="ps_b1", bufs=2, space="PSUM") as ps1,
                tc.tile_pool(name="ps_b2", bufs=2, space="PSUM") as ps2,
            ):
                def emit_p2(c0, is_sh, i):
                    # phase-2 for one 128-token tile; dequant alternates
                    # DVE/ACT so neither engine backs the PE stream up
                    tt = c0 // 128 + i
                    w2T = w2sb if is_sh else w2b
                    w2_3 = w2T[:, :].rearrange("p (hc d) -> p hc d", hc=HCB)
                    po = ps2.tile([128, 1024], dt, tag="po", name=f"po{tt}")
                    for hc2 in range(HCB // 2):
                        st, sp = hc2 == 0, hc2 == HCB // 2 - 1
                        lh = h13_3[:, 2 * hc2:2 * hc2 + 2,
                                   tt * 128:(tt + 1) * 128]
                        for j in range(2):
                            nc.tensor.matmul(
                                po[:, ts(j, 512)], lh,
                                w2_3[:, 2 * hc2:2 * hc2 + 2, ts(j, 512)],
                                start=st, stop=sp, perf_mode=DR)
                    ot = op_.tile([128, 1024], f16, tag="ot", name=f"ot{tt}")
                    if tt % 2 == 0 and tt != NTT - SHARE // 128 - 1:
                        nc.vector.tensor_scalar_mul(ot, po,
                                                    sct_b[:, tt:tt + 1])
                    else:
                        nc.scalar.activation(ot, po,
                                             mybir.ActivationFunctionType.Copy,
                                             scale=sct_b[:, tt:tt + 1])
                    nc.scalar.dma_start(out=out_r[tt], in_=ot)

                pend_p2 = []
                for (c0, cw, is_sh) in chunks:
                    w13T = w13sb if is_sh else w13b
                    p2q = list(pend_p2)
                    pend_p2 = []
                    for hc in range(HCB):
                        p1 = ps1.tile([128, 512], dt, tag="p1",
                                      name=f"p1_{c0}_{hc}")
                        p3 = ps1.tile([128, 512], dt, tag="p3",
                                      name=f"p3_{c0}_{hc}")
                        for kc2 in range(KCB // 2):
                            st, sp = kc2 == 0, kc2 == KCB // 2 - 1
                            rh = xf3[:, 2 * kc2:2 * kc2 + 2, c0:c0 + cw]
                            o1 = hc * 2048 + kc2 * 256
                            o3 = hc * 2048 + 1024 + kc2 * 256
                            nc.tensor.matmul(
                                p1[:, 0:cw],
                                w13T[:, o1:o1 + 256].rearrange(
                                    "p (two m) -> p two m", two=2),
                                rh, start=st, stop=sp, perf_mode=DR)
                            nc.tensor.matmul(
                                p3[:, 0:cw],
                                w13T[:, o3:o3 + 256].rearrange(
                                    "p (two m) -> p two m", two=2),
                                rh, start=st, stop=sp, perf_mode=DR)
                        # h13 = silu(z1) * z3_psum; z3 psum carries scale
                        # SXF*SW3 = 16 so the fp8 h13 stays in range and the
                        # dequant folds into the per-token output scale.
                        silu8 = sp_.tile([128, 512], f8, tag="silu",
                                         name=f"si_{c0}_{hc}")
                        nc.scalar.activation(silu8[:, 0:cw], p1[:, 0:cw],
                                             mybir.ActivationFunctionType.Silu,
                                             scale=scl_t[:, 0:1])
                        nc.vector.tensor_mul(
                            h13big[:, hc * TPAD + c0:hc * TPAD + c0 + cw],
                            silu8[:, 0:cw], p3[:, 0:cw])
                        # previous chunk's phase 2, pipelined into this one
                        if hc % 2 == 1 and p2q:
                            p2q.pop(0)()
                    for fn in p2q:
                        fn()
                    pend_p2 = [
                        (lambda c=c0, s=is_sh, i=i: emit_p2(c, s, i))
                        for i in range(cw // 128)]
                for fn in pend_p2:
                    fn()

    nc.compile()
    return nc


def _programs():
    if "A" not in _CACHE:
        _CACHE["A"] = _build_attn()
    if "Bp" not in _CACHE:
        _CACHE["Bp"] = _build_moe()
    return _CACHE["A"], _CACHE["Bp"]


def _run(nc, in_maps, trace=False):
    from concourse.bass_utils import run_bass_kernel_spmd
    return run_bass_kernel_spmd(nc, in_maps, list(range(NCORES)), trace=trace)


# --------------------------------------------------------------------------
# host-side orchestration
# --------------------------------------------------------------------------

def _rmsnorm(x, w):
    return x * (1.0 / np.sqrt((x * x).mean(-1, keepdims=True) + EPS)) * w


# head-contiguous layout: head h occupies partitions h*64:(h+1)*64, with
# evens in the first 32 rows and odds in the next 32 — no score masks needed.
_PERM = np.concatenate([
    np.arange(0, 64, 2), np.arange(1, 64, 2),
    64 + np.arange(0, 64, 2), 64 + np.arange(1, 64, 2),
])
_PERMS = np.concatenate([
    np.arange(1, 64, 2), np.arange(0, 64, 2),
    64 + np.arange(1, 64, 2), 64 + np.arange(0, 64, 2),
])  # even<->odd partner swap per head


def _pack_kc(w):
    # (1024, 128) -> (128, 8*128): block kc holds rows kc*128..kc*128+127
    return np.ascontiguousarray(
        w.reshape(8, 128, 128).transpose(1, 0, 2).reshape(128, 1024))


def prep_attn_inputs(x, freqs_cos, freqs_sin, att_norm_w, wq, wk, wv, wo):
    xn = _rmsnorm(x.reshape(N, D), att_norm_w)
    sx = _p2(64.0, np.abs(xn).max())
    wk_s = wk * (1.0 / np.sqrt(HD))
    swq = _p2(64.0, np.abs(wq).max())
    swk = _p2(64.0, np.abs(wk_s).max())
    swv = _p2(64.0, np.abs(wv).max())
    swo = _p2(64.0, np.abs(wo).max())
    # rope output scale folding: cos tile carries SQ8/(sx*swq); k reuses the
    # same tile so its fp8 out scale is sk8 = swk*SQ8/swq.
    sk8 = swk * SQ8 / swq
    cscale = SQ8 / (sx * swq)

    xnt8 = np.ascontiguousarray(xn.T * sx).astype(E4NP)
    cosT = np.ascontiguousarray(freqs_cos.T)    # (32, S)
    sinT = np.ascontiguousarray(freqs_sin.T)
    cos2 = (np.tile(cosT, (4, 1)) * cscale).astype(BF16NP)    # (128, S)
    sin2 = np.tile(sinT, (4, 1)) * cscale
    sin2[32:64, :] *= -1.0    # odd-dim rows (o1 = t1*c + t0*s)
    sin2[96:128, :] *= -1.0
    sin2n = np.ascontiguousarray(sin2).astype(BF16NP)

    sconst = np.zeros((128, 3), FP32)
    sconst[:, 0] = 1.0 / (SQ8 * sk8)          # exp dequant
    sconst[:, 1] = SV8 / (sx * swv)           # v requant
    sconst[:, 2] = 1.0 / (SV8 * swo)          # partial dequant

    cspack = np.ascontiguousarray(np.hstack([cos2, sin2n]))   # (128, 2S)
    xk = xnt8.reshape(8, 128, N)
    x0a = xk[0:4, :, 0:512].transpose(1, 0, 2).reshape(128, 2048)
    x0b = xk[4:8, :, 0:512].transpose(1, 0, 2).reshape(128, 2048)
    in_maps = []
    for c in range(NCORES):
        blk = slice(c * 128, (c + 1) * 128)
        wo_blk = wo[blk, :]
        wo2 = np.concatenate([wo_blk[0:64, :], wo_blk[64:128, :]],
                             axis=1) * swo   # (64, 2048)
        spk = np.hstack([
            x0a,
            _pack_kc(wq[:, blk][:, _PERM] * swq).astype(E4NP),
            _pack_kc(wq[:, blk][:, _PERMS] * swq).astype(E4NP),
            x0b,
            _pack_kc(wk_s[:, blk][:, _PERM] * swk).astype(E4NP),
            _pack_kc(wk_s[:, blk][:, _PERMS] * swk).astype(E4NP),
            _pack_kc(wv[:, blk] * swv).astype(E4NP),
        ])
        in_maps.append({
            "xnt": xnt8,
            "spk": np.ascontiguousarray(spk),
            "wo2": np.ascontiguousarray(wo2).astype(E4NP),
            "cspack": cspack,
            "onesv": np.ones((1, 64), FP32),
            "sconst": sconst,
        })
    return in_maps


def route(xf, gate_w):
    g = xf @ gate_w.T
    g = g - g.max(-1, keepdims=True)
    p = np.exp(g)
    p /= p.sum(-1, keepdims=True)
    idx = np.argsort(-p, axis=1, kind="stable")[:, :TOPK]      # (N, 2)
    vals = np.take_along_axis(p, idx, axis=1)
    w = vals / (vals.sum(-1, keepdims=True) + 1e-9)
    experts = []
    for e in range(E):
        m = idx == e
        tok = np.nonzero(m.any(1))[0]
        wt = (w * m).sum(1)[tok]
        experts.append((tok, wt.astype(FP32)))
    return experts


def _pack_w13(w, s):
    # (D, HM) -> (128, 8*1024): col = hc*1024 + kc*128 + m  (hc-major)
    return np.ascontiguousarray(
        (w * s).reshape(8, 128, 8, 128).transpose(1, 2, 0, 3).reshape(128, 8192)
    ).astype(E4NP)


def _pack_w2(w, s):
    # (HM, D) -> (128, 8*1024): col = hc*1024 + d
    return np.ascontiguousarray(
        (w * s).reshape(8, 128, D).transpose(1, 0, 2).reshape(128, 8192)
    ).astype(E4NP)


def _pack_w13x(w1, w3, s1, s3):
    # interleave the two hc-major packs per 1024-col hc block
    a = _pack_w13(w1, s1).reshape(128, 8, 1024)
    b = _pack_w13(w3, s3).reshape(128, 8, 1024)
    return np.ascontiguousarray(
        np.concatenate([a, b], axis=2).reshape(128, 16384))


def kernel(**inputs):
    ins = {k: np.ascontiguousarray(np.asarray(v)) for k, v in inputs.items()}
    x = ins["x"].astype(FP32, copy=False)
    nc_a, _ = _programs()

    # ----- launch A: attention -----
    in_maps = prep_attn_inputs(
        x, ins["freqs_cos"], ins["freqs_sin"], ins["att_norm_w"],
        ins["wq"], ins["wk"], ins["wv"], ins["wo"],
    )
    res_a = _run(nc_a, in_maps, trace=_CACHE.get("trace", False))
    _CACHE["res_a"] = res_a

    h = x.reshape(N, D).copy()
    for c in range(NCORES):
        h += res_a.results[c]["partial"].astype(FP32)

    # ----- host routing -----
    xf = _rmsnorm(h, ins["ffn_norm_w"])
    experts = route(xf, ins["gate_w"])

    max_ct = max(len(t) for t, _ in experts)
    cap = CAP if max_ct <= CAP else ((max_ct + 127) // 128) * 128
    key = f"Bp{cap}"
    if key not in _CACHE:
        _CACHE[key] = _CACHE.get("Bp") if cap == CAP else _build_moe(cap)
        if _CACHE[key] is None:
            _CACHE[key] = _build_moe(cap)
    nc_b = _CACHE[key]
    tpad = cap + SHARE

    sw1 = _p2(64.0, max(np.abs(ins["ew1"]).max(), np.abs(ins["sw1"]).max()))
    sw3 = SW3
    sw2 = _p2(64.0, max(np.abs(ins["ew2"]).max(), np.abs(ins["sw2"]).max()))
    ntt = tpad // 128
    sconst0 = np.zeros((128, 2), FP32)
    sconst0[:, 0] = 1.0 / (SXF * sw1)
    sconst0[:, 1] = 1.0 / (SXF * sw3)

    xf8 = (xf * SXF).astype(E4NP)     # (N, D)
    w13s_p = _pack_w13x(ins["sw1"], ins["sw3"], sw1, sw3)
    w2s_p = _pack_w2(ins["sw2"], sw2)

    in_maps_b = []
    for c in range(NCORES):
        tok, wt = experts[c]
        ct = len(tok)
        xft = np.zeros((D, tpad), E4NP)
        xft[:, :ct] = xf8[tok].T
        xft[:, cap:] = xf8[c * SHARE:(c + 1) * SHARE].T
        sc = np.zeros((tpad,), FP32)
        sc[:ct] = wt / (SXF * sw3 * sw2)
        sc[cap:] = 1.0 / (SXF * sw3 * sw2)
        sconst = np.hstack([sconst0, sc.reshape(ntt, 128).T]).astype(FP32)
        in_maps_b.append({
            "xft": xft,
            "w13e": _pack_w13x(ins["ew1"][c], ins["ew3"][c], sw1, sw3),
            "w2e": _pack_w2(ins["ew2"][c], sw2),
            "w13s": w13s_p,
            "w2s": w2s_p,
            "sconst": np.ascontiguousarray(sconst),
        })
    res_b = _run(nc_b, in_maps_b, trace=_CACHE.get("trace", False))
    _CACHE["res_b"] = res_b

    # ----- combine -----
    y = h.copy()
    for c in range(NCORES):
        o = res_b.results[c]["out"].astype(FP32)
        tok, _ = experts[c]
        ct = len(tok)
        y[tok] += o[:ct]
        y[c * SHARE:(c + 1) * SHARE] += o[cap:]
    return y.reshape(B, S, D).astype(ins["x"].dtype, copy=False)
